# revision 58
# baseline (speedup 1.0000x reference)
"""AttentionBlock (GroupNorm + 8-head attention + proj + residual) for
Trainium2, data-parallel over batch across 8 NeuronCores (2 batches/core).

Structure (per batch):
  h   = GroupNorm(x)          -> fp8e4 h8
  q,k = W h + b               -> fp8 in the (d%32, d//32) split layout
                                 (fp8 DoubleRow matmuls, W scaled x64)
  v^T                         -> fp8 (produced transposed; h8 stationary)
  per head pair:
    S^T = k^T q               -> fp8 DoubleRow (32-part tiles), both heads
                                 packed into one [128,1024] psum
    E   = exp(S * scale)      -> fp8e4 straight out of ScalarE, or a
                                 Schraudolph int8 fast-exp on DVE (~46% of
                                 tiles) to split the softmax-exp load
    AV  = v @ E               -> fp8 DoubleRow, ones-col accumulates the
                                 softmax denominator in-psum
    h'  = AV * (8/denom)      -> fp8 (denom rows ScalarE-copied, broadcast
                                 via a 0.125 K=1 bf16 matmul, one exact DVE
                                 reciprocal, multiply)
  out = W_p h' / 512 + b_eff + x   (fp8 DoubleRow + K=1 bf16 bias matmul)

fp8 scaling: weights stored as 64*W (keeps N(0,0.02) weights out of e4m3
denormals), h' stored as 8*h'; compensated exactly by 1/64 on the q/k/v
copies and 1/512 on the proj output.
"""

import numpy as np

import concourse.bass as bass
import concourse.tile as tile
from concourse import mybir
from concourse.bass_utils import run_bass_kernel_spmd

F32 = mybir.dt.float32
BF16 = mybir.dt.bfloat16
F8E4 = mybir.dt.float8e4
I8 = mybir.dt.int8
AF = mybir.ActivationFunctionType
ALU = mybir.AluOpType
DR = mybir.MatmulPerfMode.DoubleRow

N_CORES = 8
B, C, H, W = 16, 512, 32, 32
HW = H * W            # 1024
NH, HD = 8, 64
GROUPS = 32
GS = C // GROUPS      # 16 channels per group
EPS = 1e-5
BPC = B // N_CORES    # 2 batches per core
CT = C // 128         # 4 channel tiles
JT = HW // 128        # 8 spatial tiles (attention j)
JTP = JT // 2         # 4 j-tile pairs (DoubleRow AV)
NSL = HW // 512       # 2 moving-dim slices of 512
NPAIR = NH // 2       # 4 head pairs
SCALE = HD ** -0.5
WS = 64.0             # fp8 weight prescale
HS = 8.0              # fp8 h' prescale (folded into the r broadcast)
RS = WS * HS          # proj psum overall scale (512)
LOG2E = 1.4426950408889634
# Schraudolph fp8e4m3 exp: bits = trunc(8*log2e*x + SCH_C); SCH_C tuned for
# min softmax error (56 = e4m3 exponent bias<<3, +0.5 trunc->round, -0.46
# Schraudolph shift)
SCH_C = 56.0 - 0.46
# exp-tile engine routing: A=ScalarE exact exp, D=DVE / P=Pool Schraudolph
EXP_PATTERN = "ADADAADADAADADAADADAADADAADADAAD"


def _split_multi_waits(nc):
    """walrus's per-instruction sync-wait slots are limited (LDWEIGHTS and
    DMA DIRECT2D reject >1). Move excess waits onto a preceding NoOp on the
    same engine — the NX sequencer processes waits in stream order, so the
    semantics are unchanged."""
    n_split = 0
    for f in nc.m.functions:
        for bb in f.blocks:
            out = []
            for inst in bb.instructions:
                si = inst.sync_info
                if si is not None and si.on_wait and len(si.on_wait) > 1:
                    waits = list(si.on_wait)
                    evsem_ok = inst.engine in (
                        mybir.EngineType.PE, mybir.EngineType.SP
                    )
                    for w in waits[:-1]:
                        if evsem_ok:
                            carrier = mybir.InstEventSemaphore(
                                name=nc.get_next_instruction_name()
                            )
                        else:
                            carrier = mybir.InstDrain(
                                name=nc.get_next_instruction_name()
                            )
                        carrier.engine = inst.engine
                        carrier.debug = inst.debug
                        carrier.sync_info = mybir.SyncInfo(
                            on_wait=[w], on_update=[]
                        )
                        out.append(carrier)
                        n_split += 1
                    si.on_wait = waits[-1:]
                    inst.sync_info = si
                out.append(inst)
            bb.instructions[:] = out
    return n_split


def build_nc(split_waits=True, has_qk_bias=False, has_beff=False):
    nc = bass.Bass()
    x_in = nc.declare_dram_parameter("x_local", [BPC, C, HW], F32, isOutput=False)
    # weights pre-permuted, pre-scaled (x64) and pre-cast to fp8 on host:
    # [p, kt, o] layout ready for direct SBUF DMA
    wq8_d = nc.declare_dram_parameter("w_q8", [128, CT, C], F8E4, isOutput=False)
    wk8_d = nc.declare_dram_parameter("w_k8", [128, CT, C], F8E4, isOutput=False)
    wv8_d = nc.declare_dram_parameter("w_v8", [128, CT, C], F8E4, isOutput=False)
    wp8_d = nc.declare_dram_parameter("w_p8", [128, CT, C], F8E4, isOutput=False)
    bq_d = nc.declare_dram_parameter("b_q", [C], F32, isOutput=False)
    bk_d = nc.declare_dram_parameter("b_k", [C], F32, isOutput=False)
    beff_d = nc.declare_dram_parameter("b_eff512", [C], F32, isOutput=False)
    gam_d = nc.declare_dram_parameter("gn_gamma", [C], F32, isOutput=False)
    bet_d = nc.declare_dram_parameter("gn_beta", [C], F32, isOutput=False)
    ind_d = nc.declare_dram_parameter("gn_ind", [128, GROUPS // CT], F32, isOutput=False)
    rep_d = nc.declare_dram_parameter("gn_rep", [GROUPS // CT, 128], F32, isOutput=False)
    out_d = nc.declare_dram_parameter("out_local", [BPC, C, HW], F32, isOutput=True)

    with tile.TileContext(nc) as tc:
        with (
            tc.tile_pool(name="wpool", bufs=1) as wpool,
            tc.tile_pool(name="cpool", bufs=1) as cpool,
            tc.tile_pool(name="hpool", bufs=2) as hpool,
            tc.tile_pool(name="qkpool", bufs=2) as qkpool,
            tc.tile_pool(name="vhpool", bufs=1) as vhpool,
            tc.tile_pool(name="epool", bufs=8) as epool,
            tc.tile_pool(name="spool", bufs=6) as spool,
            tc.tile_pool(name="npool", bufs=8) as npool,
            tc.tile_pool(name="opool", bufs=6) as opool,
            tc.tile_pool(name="ps2", bufs=2, space="PSUM") as ps2,
            tc.tile_pool(name="pssp", bufs=2, space="PSUM") as pssp,
            tc.tile_pool(name="psav", bufs=1, space="PSUM") as psav,
        ):
            # x for batch 0 first: GroupNorm is the head of the critical
            # path; the weight loads only gate qkv ~10us later
            # xl split per-kt so GN kt0 starts as soon as its own DMA lands
            # (tile-granular deps would otherwise gate bn_stats on all 4)
            xl_tiles = []
            h8_tiles = []
            for b in range(BPC):
                xl_tiles.append([
                    hpool.tile([128, HW], F32, tag=f"xl{kt}", name=f"xl{b}_{kt}")
                    for kt in range(CT)
                ])
                h8_tiles.append(hpool.tile([128, CT, HW], F8E4, tag="h8",
                                           name=f"h8{b}"))
            # ---------- head DMAs ----------
            # The DMA device round-robins across the 3 issue queues, so the
            # critical-path transfers (batch-0 x for GroupNorm) are emitted
            # FIRST on each queue; constants slot in behind them.
            # batch-0 x occupies the first round-robin rounds of the serial
            # DMA device (kt0/kt3 on SP, kt1 on ACT, kt2 on Pool); weights
            # and batch-1 x follow right behind so the DVE stream's
            # DMA-gated instructions fire when the scheduler expects them
            x_dma_engs = (nc.sync, nc.scalar, nc.gpsimd, nc.sync)
            for kt in (0, 1, 2, 3):
                x_dma_engs[kt].dma_start(
                    out=xl_tiles[0][kt], in_=x_in[0, kt * 128:(kt + 1) * 128, :]
                )
            ind16 = cpool.tile([128, GROUPS // CT], F32, tag="ind16")
            nc.sync.dma_start(out=ind16, in_=ind_d.ap())
            rep_sb = cpool.tile([GROUPS // CT, 128], F32, tag="rep")
            nc.sync.dma_start(out=rep_sb, in_=rep_d.ap())
            bq_sb = cpool.tile([128, CT], F32, tag="bq")
            bk_sb = cpool.tile([128, CT], F32, tag="bk")
            gam_sb = cpool.tile([128, CT], F32, tag="gam")
            bet_sb = cpool.tile([128, CT], F32, tag="bet")
            for sb, d in ((gam_sb, gam_d), (bet_sb, bet_d)):
                nc.scalar.dma_start(out=sb, in_=d.rearrange("(m p) -> p m", p=128))
            ones_bf = cpool.tile([1, 512], BF16, tag="onesbf")
            nc.vector.memset(ones_bf, 1.0)
            # per-partition 1/RS for the proj output rescale
            rsc_sb = cpool.tile([128, 1], F32, tag="rsc")
            nc.vector.memset(rsc_sb, 1.0 / RS)
            eps_sb = cpool.tile([128, 1], F32, tag="eps")
            nc.vector.memset(eps_sb, EPS)

            # ---------- weights (loaded once, already fp8 at 64x) ----------
            # wq/wk ride SP+scalar right behind batch 0's x so q AND k are
            # ready early; wv/wp (needed later) follow; batch 1's x fills in
            # behind on spread queues.
            wq8 = wpool.tile([128, CT, C], F8E4, tag="wq8")
            wk8 = wpool.tile([128, CT, C], F8E4, tag="wk8")
            wv8 = wpool.tile([128, CT, C], F8E4, tag="wv8")
            wp8 = wpool.tile([128, CT, C], F8E4, tag="wp8")
            w_srcs = (
                (wq8, wq8_d, nc.sync),
                (wk8, wk8_d, nc.scalar),
                (wv8, wv8_d, nc.sync),
                (wp8, wp8_d, nc.scalar),
            )
            for wi, (w_sb, w_src, dq) in enumerate(w_srcs):
                dq.dma_start(out=w_sb, in_=w_src.ap())

            def x1_dma_phase():
                # batch-1 x, needed only by gn_stats(1) (~unit 3): emitted
                # after the first att unit to keep the head DMA device free
                for kt in range(CT):
                    x_dma_engs[kt].dma_start(
                        out=xl_tiles[1][kt],
                        in_=x_in[1, kt * 128:(kt + 1) * 128, :]
                    )
            for sb, d in ((bq_sb, bq_d), (bk_sb, bk_d)):
                nc.sync.dma_start(out=sb, in_=d.rearrange("(m p) -> p m", p=128))
            # proj bias row (512*b_eff), bf16, on partition 0 for the K=1
            # psum-init matmul
            befftmp = cpool.tile([1, C], F32, tag="befftmp")
            nc.sync.dma_start(out=befftmp, in_=beff_d.rearrange("(a c) -> a c", a=1))
            beff_bf = cpool.tile([1, C], BF16, tag="beffbf")
            nc.vector.tensor_copy(beff_bf, befftmp)

            # AV stationary ones/zeros columns are batch-invariant: write once.
            # Even heads ("A"): v in cols 0-63, ones col 64 -> denominator on
            # psum row 64. Odd heads ("B"): v in cols 64-127 (lane-aligned
            # with final destination), ones col 32 -> denominator on row 32.
            # merged AV stationary: cols 0-63 v_a, col 64 ones (A denom on
            # psum row 64); cols 65-192 are the B stationary: B-col 32 (=97)
            # ones -> B denom on psum row 32, B-cols 64-127 (=129-192) v_b
            # dual-fp8 LDWEIGHTS needs even column offsets/widths: A block
            # cols 0:66 (64 v + ones col 64 + zero pad), B block at 66:194
            # (ones at 98 -> B denom row 32, v_b at 130:194 -> rows 64-127)
            vh_t = vhpool.tile([128, JTP, 2, NPAIR, 196], F8E4, tag="vh")
            nc.vector.memset(vh_t[:, :, :, :, 64:65], 1.0)
            nc.gpsimd.memset(vh_t[:, :, :, :, 65:132], 0.0)
            nc.gpsimd.memset(vh_t[:, :, :, :, 100:101], 1.0)
            # denominator-broadcast selector: K=1 row of 0.125 (recip of
            # denom/8 bakes the fp8 h' prescale of 8 into r)
            sel_bf = cpool.tile([1, 64], BF16, tag="selbf")
            nc.vector.memset(sel_bf, 1.0 / HS)


            # ---------- phase closures ----------
            # Engine instruction streams execute in program order, so batch
            # phases are hand-interleaved below: batch 1's GroupNorm/qkv are
            # emitted in the middle of batch 0's attention units, keeping
            # every engine fed across the batch boundary.
            q_tiles, k_tiles, hav_tiles = {}, {}, {}
            exp_ctr = [0]

            gn_ab = {}

            def gn_stats(b):
                # combined all-kt stats chain: one pass of bn_stats (kt in
                # DMA-arrival order), one group matmul, one scalar chain and
                # one broadcast matmul for all 4 kt. Produces gn_ab[b] with
                # per-(channel, kt) scale/shift columns.
                xl_t = xl_tiles[b]
                st = spool.tile([128, CT, 2, 6], F32, tag="bnst")
                s3 = spool.tile([128, CT, 3], F32, tag="s3")
                for kt in range(CT):
                    for s in range(2):
                        nc.vector.bn_stats(
                            out=st[:, kt, s, :],
                            in_=xl_t[kt][:, s * 512:(s + 1) * 512]
                        )
                    nc.vector.bn_aggr(out=s3[:, kt, 0:2], in_=st[:, kt])
                nc.gpsimd.tensor_mul(s3[:, :, 2:3], s3[:, :, 0:1], s3[:, :, 0:1])
                gps = ps2.tile([128, 512], F32, tag="ps2t")
                nc.tensor.matmul(
                    gps[0:8, 0:3 * CT],
                    lhsT=ind16, rhs=s3.rearrange("p kt c -> p (kt c)"),
                    start=True, stop=True,
                )
                # g3[:, kt, :]: group-mean of (mean, var, mean^2) per kt
                g3 = spool.tile([8, CT, 3], F32, tag="g3")
                nc.vector.tensor_copy(g3.rearrange("p kt c -> p (kt c)"),
                                      gps[0:8, 0:3 * CT])
                mr = spool.tile([8, 2, CT], F32, tag="mr")
                vg = spool.tile([8, 2, CT], F32, tag="vg")
                nc.gpsimd.tensor_copy(mr[:, 0, :], g3[:, :, 0])
                nc.gpsimd.tensor_add(vg[:, 0, :], g3[:, :, 1], g3[:, :, 2])
                nc.gpsimd.tensor_mul(vg[:, 1, :], g3[:, :, 0], g3[:, :, 0])
                nc.gpsimd.tensor_sub(vg[:, 0, :], vg[:, 0, :], vg[:, 1, :])
                # rstd = exp(-0.5*ln(var+eps)): stays in the
                # natural_log_exp ACT table set
                nc.scalar.activation(
                    out=vg[:, 1, :], in_=vg[:, 0, :], func=AF.Ln,
                    bias=eps_sb[0:8, :], scale=1.0,
                )
                nc.scalar.activation(
                    out=mr[:, 1, :], in_=vg[:, 1, :], func=AF.Exp,
                    scale=-0.5,
                )
                bc = ps2.tile([128, 512], F32, tag="ps2t")
                nc.tensor.matmul(
                    bc[0:128, 0:2 * CT],
                    lhsT=rep_sb, rhs=mr.rearrange("p a kt -> p (a kt)"),
                    start=True, stop=True,
                )
                # ab[:, 0, kt] = rstd*gamma; ab[:, 1, kt] = beta - mean*that
                ab = spool.tile([128, 3, CT], F32, tag="ab")
                # bc is PSUM: GPSIMD has no PSUM port, keep these on DVE
                nc.vector.tensor_mul(ab[:, 0, :], bc[:, CT:2 * CT], gam_sb)
                nc.vector.tensor_mul(ab[:, 2, :], bc[:, 0:CT], ab[:, 0, :])
                nc.vector.tensor_sub(ab[:, 1, :], bet_sb, ab[:, 2, :])
                gn_ab[b] = ab

            def gn_apply(b, kt, eng):
                ab, h8_t = gn_ab[b], h8_tiles[b]
                if eng == "A":
                    nc.scalar.activation(
                        out=h8_t[:, kt, :], in_=xl_tiles[b][kt],
                        func=AF.Identity,
                        bias=ab[:, 1, kt:kt + 1], scale=ab[:, 0, kt:kt + 1],
                    )
                else:
                    nc.gpsimd.tensor_scalar(
                        out=h8_t[:, kt, :], in0=xl_tiles[b][kt],
                        scalar1=ab[:, 0, kt:kt + 1], scalar2=ab[:, 1, kt:kt + 1],
                        op0=ALU.mult, op1=ALU.add,
                    )

            def gn_phase(b, applies="APAP"):
                gn_stats(b)
                for kt in range(CT):
                    gn_apply(b, kt, applies[kt])

            def ensure_qk(b):
                if b not in q_tiles:
                    q_tiles[b] = qkpool.tile([128, 2, 2, HW], F8E4, tag="q",
                                             name=f"q{b}")
                    k_tiles[b] = qkpool.tile([128, 2, 2, HW], F8E4, tag="k",
                                             name=f"k{b}")

            def qk_unit(b, m, wi, isl, ceng="A"):
                # one (m, q-or-k, isl) granule: 2 DR matmuls + psum drain.
                # q8/k8 layout for the DoubleRow S matmul: head h lives on
                # partitions 32*(h%4)..+32 of group g=h//4, with d split as
                # (d%32 -> partition, d//32 -> free dim). The m-th psum tile
                # holds (g=m//2, d_hi=m%2) via host-side weight column
                # permutation.
                ensure_qk(b)
                h8_t = h8_tiles[b]
                w_sb, b_sb, dst = (
                    (wq8, bq_sb, q_tiles[b]), (wk8, bk_sb, k_tiles[b]),
                )[wi]
                pq = ps2.tile([128, 512], F32, tag="ps2t")
                for tp in range(CT // 2):
                    nc.tensor.matmul(
                        pq[:, :],
                        lhsT=w_sb[:, 2 * tp:2 * tp + 2,
                                  m * 128:(m + 1) * 128],
                        rhs=h8_t[:, 2 * tp:2 * tp + 2,
                                 isl * 512:(isl + 1) * 512],
                        start=(tp == 0), stop=(tp == CT // 2 - 1),
                        perf_mode=DR,
                    )
                if has_qk_bias:
                    nc.vector.tensor_scalar(
                        out=dst[:, m // 2, m % 2, isl * 512:(isl + 1) * 512],
                        in0=pq[:, :],
                        scalar1=1.0 / WS, scalar2=b_sb[:, m:m + 1],
                        op0=ALU.mult, op1=ALU.add,
                    )
                elif ceng == "D":
                    nc.vector.tensor_scalar(
                        out=dst[:, m // 2, m % 2, isl * 512:(isl + 1) * 512],
                        in0=pq[:, :], scalar1=1.0 / WS, scalar2=None,
                        op0=ALU.mult,
                    )
                else:
                    nc.scalar.activation(
                        out=dst[:, m // 2, m % 2, isl * 512:(isl + 1) * 512],
                        in_=pq[:, :], func=AF.Copy,
                        scale=1.0 / WS,
                    )

            def qk_phase0_head():
                # batch-0 m0/m1 q+k in first-S-need order (isl0 first),
                # copies alternating ACT/DVE: both engines are idle here and
                # the first S matmul gates the whole attention pipeline
                for i, (m, wi, isl) in enumerate((
                    (0, 0, 0), (0, 1, 0), (1, 0, 0), (1, 1, 0),
                    (0, 1, 1), (1, 1, 1), (0, 0, 1), (1, 0, 1),
                )):
                    qk_unit(0, m, wi, isl, ceng="AD"[i % 2])

            def qk_granules(b, ms):
                return [
                    (lambda b=b, m=m, wi=wi, isl=isl: qk_unit(b, m, wi, isl))
                    for m in ms for wi in range(2) for isl in range(NSL)
                ]

            def v_phase(b, mjs):
                # v, produced transposed ([j, o]) with h8 as the stationary
                h8_t = h8_tiles[b]
                for mj in list(mjs):
                    pv = ps2.tile([128, 512], F32, tag="ps2t")
                    for tp in range(CT // 2):
                        nc.tensor.matmul(
                            pv[:, 0:512],
                            lhsT=h8_t[:, 2 * tp:2 * tp + 2,
                                      mj * 128:(mj + 1) * 128],
                            rhs=wv8[:, 2 * tp:2 * tp + 2, :],
                            start=(tp == 0), stop=(tp == CT // 2 - 1),
                            perf_mode=DR,
                        )
                    pv_h = pv[:, 0:512].rearrange(
                        "p (hp a d) -> p hp a d", hp=NPAIR, a=2
                    )
                    nc.scalar.activation(
                        out=vh_t[:, mj // 2, mj % 2, :, 0:64],
                        in_=pv_h[:, :, 0, :], func=AF.Copy, scale=1.0 / WS,
                    )
                    nc.scalar.activation(
                        out=vh_t[:, mj // 2, mj % 2, :, 132:196],
                        in_=pv_h[:, :, 1, :], func=AF.Copy, scale=1.0 / WS,
                    )

            def att_core(b, hp, isl, fill=None, pre_av=None):
                # one (head pair, i-half), restructured as an S/exp BLOCK
                # followed by an AV block: the previous unit's normalization
                # tail (pre_av) slots between them, so its broadcast matmuls
                # never head-of-line-block the in-order PE stream, and its
                # reciprocal/muls drain on DVE while this unit's S block
                # still runs — by the time the AV block needs the av psum
                # slots they are free.
                if b not in hav_tiles:
                    # attention output, fp8 at 8x (the 8 comes from the
                    # denominator broadcast at denom/8)
                    hav_tiles[b] = hpool.tile([128, NPAIR, HW], F8E4,
                                              tag="h8av", name=f"h8av{b}")
                q_t, k_t, h8av = q_tiles[b], k_tiles[b], hav_tiles[b]
                sl = slice(isl * 512, (isl + 1) * 512)
                e8s = []
                for jbp in range(JTP):
                    # both heads' E in one tile: A in cols 0:512, B 512:1024
                    e8 = epool.tile([128, 2, 1024], F8E4, tag="e8")
                    e8s.append(e8)
                    for js in range(2):
                        jb = 2 * jbp + js
                        # both heads' S^T packed in one 2-bank psum so one
                        # exp op covers the pair
                        pss = pssp.tile([128, 1024], F32, tag="pss")
                        for a in range(2):
                            h = 2 * hp + a
                            g, base = h // 4, 32 * (h % 4)
                            nc.tensor.matmul(
                                pss[:, a * 512:(a + 1) * 512],
                                lhsT=k_t[base:base + 32, g, :,
                                         jb * 128:(jb + 1) * 128],
                                rhs=q_t[base:base + 32, g, :, sl],
                                start=True, stop=True,
                                perf_mode=DR,
                                tile_position=(base, 0),
                            )
                        eng = EXP_PATTERN[exp_ctr[0] % len(EXP_PATTERN)]
                        exp_ctr[0] += 1
                        if eng == "A":
                            nc.scalar.activation(
                                out=e8[:, js, :], in_=pss[:, :],
                                func=AF.Exp, scale=SCALE,
                            )
                        else:
                            # Schraudolph fast exp straight into fp8e4 bits:
                            # round(8*log2e*scale*S + SCH_C) on DVE (GPSIMD
                            # cannot read PSUM)
                            nc.vector.tensor_scalar(
                                out=e8.bitcast(I8)[:, js, :],
                                in0=pss[:, :],
                                scalar1=8.0 * LOG2E * SCALE,
                                scalar2=SCH_C,
                                op0=ALU.mult, op1=ALU.add,
                            )
                    if jbp == 0 and pre_av is not None:
                        pre_av()
                    elif fill is not None:
                        fill()
                avA = psav.tile([68, 512], F32, tag="avA")
                avB = psav.tile([128, 512], F32, tag="avB")
                for jbp in range(JTP):
                    # AV accumulation (unnormalized, DoubleRow over j-tile
                    # pairs); ones columns accumulate softmax denominators
                    # on avA row 64 / avB row 32
                    nc.tensor.matmul(
                        avA[:, :], lhsT=vh_t[:, jbp, :, hp, 0:68],
                        rhs=e8s[jbp][:, :, 0:512],
                        start=(jbp == 0), stop=(jbp == JTP - 1),
                        perf_mode=DR,
                    )
                    nc.tensor.matmul(
                        avB[:, :], lhsT=vh_t[:, jbp, :, hp, 68:196],
                        rhs=e8s[jbp][:, :, 512:1024],
                        start=(jbp == 0), stop=(jbp == JTP - 1),
                        perf_mode=DR,
                    )
                if fill is not None:
                    fill()
                return avA, avB

            def att_norm_head(b, hp, isl, avA, avB):
                # denominator rows -> bf16 SBUF (ScalarE + DVE), emitted
                # right after the unit's AV block
                dbfA = npool.tile([1, 512], BF16, tag="dbfA")
                dbfB = npool.tile([1, 512], BF16, tag="dbfB")
                nc.scalar.activation(out=dbfA, in_=avA[64:65, :], func=AF.Copy)
                nc.vector.tensor_copy(dbfB, avB[32:33, :])
                return dbfA, dbfB

            def att_norm_tail(b, hp, isl, avA, avB, dbfA, dbfB):
                # normalize: h' = 8*av/denom, via a 0.125-valued K=1 bf16
                # broadcast matmul (plain mode: DoubleRow can't target dst
                # partition 64), one exact DVE reciprocal, then a multiply.
                # Runs as the NEXT unit's pre_av (software pipelining).
                h8av = hav_tiles[b]
                sl = slice(isl * 512, (isl + 1) * 512)
                Dp = ps2.tile([128, 512], F32, tag="ps2t")
                nc.tensor.matmul(
                    Dp[0:64, :], lhsT=sel_bf, rhs=dbfA,
                    start=True, stop=True,
                )
                nc.tensor.matmul(
                    Dp[64:128, :], lhsT=sel_bf, rhs=dbfB,
                    start=True, stop=True,
                )
                rcp = npool.tile([128, 512], F32, tag="rcp")
                nc.vector.reciprocal(out=rcp, in_=Dp[:, :])
                nc.vector.tensor_mul(h8av[0:64, hp, sl], avA[0:64, :],
                                     rcp[0:64, :])
                nc.vector.tensor_mul(h8av[64:128, hp, sl], avB[64:128, :],
                                     rcp[64:128, :])

            proj_ctr = [0]

            def proj_unit(b, m, isl, drain="auto"):
                # one (m, isl) output granule: 2 DR matmuls, residual-add
                # drain, and its own [128,512] out DMA so the tail drains
                # incrementally instead of in whole-m chunks
                xl_t, h8av = xl_tiles[b], hav_tiles[b]
                sl = slice(isl * 512, (isl + 1) * 512)
                po = ps2.tile([128, 512], F32, tag="ps2t")
                if has_beff:
                    # general path: bias row seeds the psum (mixing
                    # bf16 into the fp8-DR group)
                    nc.tensor.matmul(
                        po[:, :],
                        lhsT=beff_bf[:, m * 128:(m + 1) * 128],
                        rhs=ones_bf[:, :],
                        start=True, stop=False,
                    )
                for tp in range(CT // 2):
                    nc.tensor.matmul(
                        po[:, :],
                        lhsT=wp8[:, 2 * tp:2 * tp + 2,
                                 m * 128:(m + 1) * 128],
                        rhs=h8av[:, 2 * tp:2 * tp + 2, sl],
                        start=(tp == 0 and not has_beff),
                        stop=(tp == CT // 2 - 1),
                        perf_mode=DR,
                    )
                ot = opool.tile([128, 512], F32, tag="ot")
                if drain == "auto" and proj_ctr[0] % 2:
                    # ACT drains the psum (scale only), Pool adds the
                    # residual: keeps the proj drain off the busy DVE
                    osc = opool.tile([128, 512], F32, tag="osc")
                    nc.scalar.activation(
                        out=osc, in_=po[:, :], func=AF.Copy, scale=1.0 / RS,
                    )
                    nc.gpsimd.tensor_add(ot, osc, xl_t[m][:, sl])
                else:
                    nc.vector.scalar_tensor_tensor(
                        out=ot, in0=po[:, :], scalar=rsc_sb,
                        in1=xl_t[m][:, sl], op0=ALU.mult, op1=ALU.add,
                    )
                dma_eng = nc.scalar if proj_ctr[0] % 2 else nc.sync
                proj_ctr[0] += 1
                dma_eng.dma_start(
                    out=out_d[b, m * 128:(m + 1) * 128, sl], in_=ot
                )

            def proj_granules(b, ms, isls, drain="auto"):
                return [
                    (lambda b=b, m=m, isl=isl: proj_unit(b, m, isl, drain))
                    for isl in isls for m in ms
                ]

            # ---------- hand-interleaved, software-pipelined schedule ----
            # units u = (b, hp, isl) run core(u_n) ... norm(u_{n-1}) so the
            # PE stream never waits for a denominator. Other-phase work is
            # fed in as fine GRANULES, one per jbp iteration inside
            # att_core, so no engine sees a multi-us burst of foreign work.
            # Batch 1 runs isl-OUTER so its isl0 projection can start while
            # the isl1 attention units are still in flight.
            units = ([(0, hp, isl) for hp in range(NPAIR)
                      for isl in range(NSL)] +
                     [(1, hp, isl) for isl in range(NSL)
                      for hp in range(NPAIR)])
            def v_gran(b, mjs):
                return lambda: v_phase(b, mjs)

            # per-unit granule lists (emitted one per jbp inside the core);
            # v pairs land one jbp ahead of the AV that reads them
            unit_fill = {
                0: [v_gran(0, (2, 3)), v_gran(0, (4, 5)), v_gran(0, (6, 7)),
                    lambda: x1_dma_phase()],
                1: qk_granules(0, (2,)),
                2: qk_granules(0, (3,)),
                3: [lambda: gn_stats(1), lambda: gn_apply(1, 0, "P")],
                4: [lambda: gn_apply(1, 1, "P"), lambda: gn_apply(1, 2, "P"),
                    lambda: gn_apply(1, 3, "P")],
                5: qk_granules(1, (0,)),
                6: qk_granules(1, (1,)),
                7: qk_granules(1, (2,)),
                8: [v_gran(1, (2, 3)), v_gran(1, (4, 5)), v_gran(1, (6, 7))],
                9: qk_granules(1, (3,)),
                10: proj_granules(0, (0, 1, 2), (0,)),
                11: proj_granules(0, (0, 1, 2), (1,)),
                12: (proj_granules(0, (3,), (0, 1)) +
                     proj_granules(1, (0,), (0,))),
                # batch-1 isl0 h'av complete after norm(u11): its projection
                # drains during the isl1 units
                13: proj_granules(1, (1, 2, 3), (0,)),
            }
            # block extras emitted between core(u_i) and core(u_{i+1})
            block_extras = {
                # vh is single-buffered: batch 1's first v pair MUST follow
                # the last batch-0 attention core (program order on PE); the
                # rest arrive as unit-8 fillers one jbp ahead of their reader
                7: lambda: v_phase(1, (0, 1)),
            }
            gn_phase(0)
            qk_phase0_head()
            v_phase(0, (0, 1))
            pending = None
            for i, (b, hp, isl) in enumerate(units):
                gq = list(unit_fill.get(i, []))
                gq.reverse()

                def fill(q=gq):
                    if q:
                        q.pop()()

                pre = (lambda p=pending: att_norm_tail(*p)) if pending else None
                av = att_core(b, hp, isl, fill=fill, pre_av=pre)
                dbf = att_norm_head(b, hp, isl, *av)
                pending = (b, hp, isl, *av, *dbf)
                while gq:
                    gq.pop()()
                if i in block_extras:
                    block_extras[i]()
            att_norm_tail(*pending)
            # kernel tail: everything else is drained, keep the last four
            # output granules on the fast DVE path
            for g in proj_granules(1, (0, 1, 2, 3), (1,), drain="D"):
                g()
    if split_waits:
        _split_multi_waits(nc)
    return nc


_NC_CACHE = {}


def _get_nc(has_qk_bias=False, has_beff=False):
    key = ("nc", has_qk_bias, has_beff)
    if key not in _NC_CACHE:
        _NC_CACHE[key] = build_nc(has_qk_bias=has_qk_bias, has_beff=has_beff)
    return _NC_CACHE[key]


def make_in_maps(x, gn_gamma, gn_beta, w_qkv, b_qkv, w_proj, b_proj):
    f = np.float32
    x = np.ascontiguousarray(np.asarray(x, dtype=f)).reshape(B, C, HW)
    w_qkvT = np.ascontiguousarray(np.asarray(w_qkv, dtype=f).T)
    w_projT = np.ascontiguousarray(np.asarray(w_proj, dtype=f).T)
    b_qkv = np.asarray(b_qkv, dtype=f)
    # q/k output-channel permutation for the DoubleRow S layout: psum tile
    # X=(g, d_hi), partition p=(h%4)*32 + d%32 holds channel
    # c=(4g + p//32)*64 + d_hi*32 + p%32
    perm = np.empty(C, dtype=np.int64)
    for X in range(CT):
        g, d_hi = X // 2, X % 2
        for p in range(128):
            perm[X * 128 + p] = (4 * g + p // 32) * 64 + d_hi * 32 + p % 32
    f8 = mybir.dt.np(F8E4)

    def to_dev_w8(wT_block):
        # device layout [p, kt, o] of W^T[(kt*128+p), o], prescaled x64, fp8
        return np.ascontiguousarray(
            (wT_block.reshape(CT, 128, -1).transpose(1, 0, 2) * WS).astype(f8)
        )

    w_q8 = to_dev_w8(w_qkvT[:, 0:C][:, perm])
    w_k8 = to_dev_w8(w_qkvT[:, C:2 * C][:, perm])
    w_v8 = to_dev_w8(w_qkvT[:, 2 * C:3 * C])
    w_p8 = to_dev_w8(w_projT)
    b_q = np.ascontiguousarray(b_qkv[0:C][perm])
    b_k = np.ascontiguousarray(b_qkv[C:2 * C][perm])
    b_v = b_qkv[2 * C:3 * C]
    # softmax rows sum to 1, so v's bias passes straight through attention:
    # fold it into the projection bias. Stored at 512x (the proj psum scale).
    b_eff512 = np.ascontiguousarray(
        RS * (np.asarray(w_proj, dtype=f) @ b_v + np.asarray(b_proj, dtype=f))
    )
    gn_gamma = np.ascontiguousarray(np.asarray(gn_gamma, dtype=f))
    gn_beta = np.ascontiguousarray(np.asarray(gn_beta, dtype=f))
    n_gpt = GROUPS // CT   # groups per 128-channel tile
    gn_ind = np.zeros((128, n_gpt), dtype=f)
    gn_rep = np.zeros((n_gpt, 128), dtype=f)
    for g in range(n_gpt):
        gn_ind[g * GS:(g + 1) * GS, g] = 1.0 / GS
        gn_rep[g, g * GS:(g + 1) * GS] = 1.0
    in_maps = []
    for c in range(N_CORES):
        in_maps.append({
            "x_local": np.ascontiguousarray(x[c * BPC:(c + 1) * BPC]),
            "w_q8": w_q8,
            "w_k8": w_k8,
            "w_v8": w_v8,
            "w_p8": w_p8,
            "b_q": b_q,
            "b_k": b_k,
            "b_eff512": b_eff512,
            "gn_gamma": gn_gamma,
            "gn_beta": gn_beta,
            "gn_ind": gn_ind,
            "gn_rep": gn_rep,
        })
    return in_maps


def kernel(x, gn_gamma, gn_beta, w_qkv, b_qkv, w_proj, b_proj):
    b_qkv_a = np.asarray(b_qkv)
    has_qk_bias = bool(np.any(b_qkv_a[0:2 * C]))
    has_beff = bool(np.any(b_qkv_a[2 * C:])) or bool(np.any(np.asarray(b_proj)))
    nc = _get_nc(has_qk_bias, has_beff)
    in_maps = make_in_maps(x, gn_gamma, gn_beta, w_qkv, b_qkv, w_proj, b_proj)
    res = run_bass_kernel_spmd(nc, in_maps, list(range(N_CORES)))
    out = np.empty((B, C, HW), dtype=np.float32)
    for c in range(N_CORES):
        out[c * BPC:(c + 1) * BPC] = res.results[c]["out_local"]
    return out.reshape(B, C, H, W)

